# revision 9
# baseline (speedup 1.0000x reference)
"""CBOW negative-sampling loss kernel for 8 trn2 NeuronCores.

Strategy (data-parallel over batch):
  - Host concatenates W_target/W_context into one bf16 table [2V, D] and
    builds per-batch-element combined row indices [B, 17]
    (target, context+V, neg_0+V..neg_14+V).
  - Each core handles B/8 = 16384 batch elements, 128 tiles of 128.
  - Per tile: one indirect (gather) DMA pulls 17*128 rows of 256B from HBM
    into SBUF with batch on partitions; DVE computes
    emb_in = emb_t * mask, prods = emb_in * emb_j, tree-folds the 128-d
    segments, reduces to 16 scores; ACT computes ln(sigmoid(-x)) with a
    fused per-partition accumulation (= -softplus(x) summed over j).
  - Final: per-core [128,1] f32 partial sums -> host sum -> loss.
"""

import os

import numpy as np
import ml_dtypes

import concourse.bass as bass
import concourse.mybir as mybir
import concourse.tile as tile
from concourse import bacc, bass_utils

V, D, B, NEGS = 100000, 128, 131072, 15
NCORES = 8
BLOC = B // NCORES  # 16384
P = 128
T = BLOC // P  # 128 tiles per core
J = 2 + NEGS  # 17 gathered rows per batch element
G = 4  # tiles per gather call

BF16 = mybir.dt.bfloat16
F32 = mybir.dt.float32
NPBF16 = ml_dtypes.bfloat16

_CACHE = {}
LAST_RESULT = None  # BassKernelResults of the most recent run (for profiling)


def _build_nc(V=V, T=T, G=G):
    nc = bacc.Bacc("TRN2", target_bir_lowering=False, debug=False)
    w = nc.dram_tensor("w_cat", [2 * V, D], BF16, kind="ExternalInput")
    idx = nc.dram_tensor("idx", [P, T * J], mybir.dt.int32, kind="ExternalInput")
    mask = nc.dram_tensor("maskr", [P, T * D], BF16, kind="ExternalInput")
    out = nc.dram_tensor("out", [P, 1], F32, kind="ExternalOutput")

    with tile.TileContext(nc) as tc:
        with (
            tc.tile_pool(name="const", bufs=1) as constp,
            tc.tile_pool(name="gather", bufs=3) as gatherp,
            tc.tile_pool(name="work", bufs=3) as workp,
            tc.tile_pool(name="small", bufs=4) as smallp,
        ):
            idx_sb = constp.tile([P, T * J], mybir.dt.int32)
            nc.sync.dma_start(idx_sb[:], idx[:])
            mask_sb = constp.tile([P, T * D], BF16)
            nc.sync.dma_start(mask_sb[:], mask[:])
            tsum = constp.tile([P, T], F32)
            scores_all = constp.tile([P, T * 16], F32)

            for g in range(T // G):
                emb = gatherp.tile([P, G * J * D], BF16, tag="emb")
                nc.gpsimd.indirect_dma_start(
                    out=emb[:],
                    out_offset=None,
                    in_=w[:],
                    in_offset=bass.IndirectOffsetOnAxis(
                        ap=idx_sb[:, g * G * J : (g + 1) * G * J], axis=0
                    ),
                )
                for k in range(G):
                    t = g * G + k
                    base = k * J * D
                    embt = emb[:, base : base + J * D]
                    emb_in = smallp.tile([P, D], BF16, tag="embin")
                    nc.vector.tensor_mul(
                        emb_in[:], embt[:, 0:D], mask_sb[:, t * D : (t + 1) * D]
                    )
                    prods = workp.tile([P, 16 * D], BF16, tag="prods")
                    p3 = prods[:].rearrange("p (j d) -> p j d", d=D)
                    nc.vector.tensor_tensor(
                        out=p3,
                        in0=embt[:, D : J * D].rearrange("p (j d) -> p j d", d=D),
                        in1=emb_in[:].unsqueeze(1).broadcast_to((P, 16, D)),
                        op=mybir.AluOpType.mult,
                    )
                    f1 = workp.tile([P, 16 * 64], BF16, tag="f1")
                    f1v = f1[:].rearrange("p (j d) -> p j d", d=64)
                    nc.vector.tensor_add(f1v, p3[:, :, 0:64], p3[:, :, 64:128])
                    f2 = workp.tile([P, 16 * 32], BF16, tag="f2")
                    f2v = f2[:].rearrange("p (j d) -> p j d", d=32)
                    nc.vector.tensor_add(f2v, f1v[:, :, 0:32], f1v[:, :, 32:64])
                    f3 = workp.tile([P, 16 * 16], BF16, tag="f3")
                    f3v = f3[:].rearrange("p (j d) -> p j d", d=16)
                    nc.vector.tensor_add(f3v, f2v[:, :, 0:16], f2v[:, :, 16:32])
                    scores = scores_all[:, t * 16 : (t + 1) * 16]
                    nc.vector.tensor_reduce(
                        scores, f3v, axis=mybir.AxisListType.X, op=mybir.AluOpType.add
                    )
                    sig = smallp.tile([P, 16], F32, tag="sig")
                    nc.scalar.activation(
                        sig[:],
                        scores,
                        mybir.ActivationFunctionType.Sigmoid,
                        scale=-1.0,
                    )
                    lnout = smallp.tile([P, 16], F32, tag="lnout")
                    nc.scalar.activation(
                        lnout[:],
                        sig[:],
                        mybir.ActivationFunctionType.Ln,
                        accum_out=tsum[:, t : t + 1],
                    )

            total = constp.tile([P, 1], F32)
            nc.vector.tensor_reduce(
                total[:], tsum[:], axis=mybir.AxisListType.X, op=mybir.AluOpType.add
            )
            nc.sync.dma_start(out[:], total[:])
    nc.compile()
    return nc


def _get_nc():
    if "nc" not in _CACHE:
        _CACHE["nc"] = _build_nc()
    return _CACHE["nc"]


def kernel(target, context, neg_idx, dropout_mask, W_target, W_context):
    global LAST_RESULT
    nc = _get_nc()

    target = np.asarray(target).astype(np.int32, copy=False)
    context = np.asarray(context).astype(np.int32, copy=False)
    neg_idx = np.asarray(neg_idx).astype(np.int32, copy=False)
    dropout_mask = np.asarray(dropout_mask, dtype=np.float32)
    W_target = np.asarray(W_target, dtype=np.float32)
    W_context = np.asarray(W_context, dtype=np.float32)

    w_cat = np.ascontiguousarray(
        np.concatenate([W_target, W_context], axis=0).astype(NPBF16)
    )
    idx_cat = np.empty((B, J), np.int32)
    idx_cat[:, 0] = target
    idx_cat[:, 1] = context + V
    idx_cat[:, 2:] = neg_idx + V
    mask_bf = dropout_mask.astype(NPBF16)

    in_maps = []
    for c in range(NCORES):
        sl = slice(c * BLOC, (c + 1) * BLOC)
        idxs = np.ascontiguousarray(
            idx_cat[sl].reshape(T, P, J).transpose(1, 0, 2).reshape(P, T * J)
        )
        maskr = np.ascontiguousarray(
            mask_bf[sl].reshape(T, P, D).transpose(1, 0, 2).reshape(P, T * D)
        )
        in_maps.append({"w_cat": w_cat, "idx": idxs, "maskr": maskr})

    trace = bool(int(os.environ.get("KERNEL_TRACE", "0")))
    res = bass_utils.run_bass_kernel_spmd(
        nc, in_maps, core_ids=list(range(NCORES)), trace=trace
    )
    LAST_RESULT = res

    tot = 0.0
    for r in res.results:
        tot += float(r["out"].astype(np.float64).sum())
    # device accumulated sum of ln(sigmoid(-x)) = -sum of softplus(x)
    loss = -tot / B
    return np.asarray(np.float32(loss))


# revision 11
# speedup vs baseline: 1.0090x; 1.0090x over previous
"""CBOW negative-sampling loss kernel for 8 trn2 NeuronCores.

Strategy (data-parallel over batch):
  - Host concatenates W_target/W_context into one bf16 table [2V, D] and
    builds per-batch-element combined row indices [B, 17]
    (target, context+V, neg_0+V..neg_14+V).
  - Each core handles B/8 = 16384 batch elements, 128 tiles of 128.
  - Per tile: one indirect (gather) DMA pulls 17*128 rows of 256B from HBM
    into SBUF with batch on partitions; DVE computes
    emb_in = emb_t * mask, prods = emb_in * emb_j, tree-folds the 128-d
    segments, reduces to 16 scores; ACT computes ln(sigmoid(-x)) with a
    fused per-partition accumulation (= -softplus(x) summed over j).
  - Final: per-core [128,1] f32 partial sums -> host sum -> loss.
"""

import os

import numpy as np
import ml_dtypes

import concourse.bass as bass
import concourse.mybir as mybir
import concourse.tile as tile
from concourse import bacc, bass_utils

V, D, B, NEGS = 100000, 128, 131072, 15
NCORES = 8
BLOC = B // NCORES  # 16384
P = 128
T = BLOC // P  # 128 tiles per core
J = 2 + NEGS  # 17 gathered rows per batch element
G = 4  # tiles per gather call

BF16 = mybir.dt.bfloat16
F32 = mybir.dt.float32
NPBF16 = ml_dtypes.bfloat16

_CACHE = {}
LAST_RESULT = None  # BassKernelResults of the most recent run (for profiling)


def _build_nc(V=V, T=T, G=G):
    nc = bacc.Bacc("TRN2", target_bir_lowering=False, debug=False)
    w = nc.dram_tensor("w_cat", [2 * V, D], BF16, kind="ExternalInput")
    idx = nc.dram_tensor("idx", [P, T * J], mybir.dt.int32, kind="ExternalInput")
    mask = nc.dram_tensor("maskr", [P, T * D], BF16, kind="ExternalInput")
    out = nc.dram_tensor("out", [P, 1], F32, kind="ExternalOutput")

    with tile.TileContext(nc) as tc:
        with (
            tc.tile_pool(name="const", bufs=1) as constp,
            tc.tile_pool(name="gather", bufs=3) as gatherp,
            tc.tile_pool(name="work", bufs=3) as workp,
            tc.tile_pool(name="small", bufs=4) as smallp,
        ):
            idx_sb = constp.tile([P, T * J], mybir.dt.int32)
            nc.sync.dma_start(idx_sb[:], idx[:])
            mask_sb = constp.tile([P, T * D], BF16)
            nc.sync.dma_start(mask_sb[:], mask[:])
            # ACT function tables: sigmoid and ln live in different table
            # sets (1283ns reload on switch), so run all sigmoids in the main
            # loop and one ln+accumulate pass at the end.
            LN_CHUNKS = 8
            tsum = constp.tile([P, LN_CHUNKS], F32)
            scores_all = constp.tile([P, T * 16], F32)
            sig_all = constp.tile([P, T * 16], F32)
            ln_scratch = constp.tile([P, T * 16 // LN_CHUNKS], F32)

            for g in range(T // G):
                emb = gatherp.tile([P, G * J * D], BF16, tag="emb")
                nc.gpsimd.indirect_dma_start(
                    out=emb[:],
                    out_offset=None,
                    in_=w[:],
                    in_offset=bass.IndirectOffsetOnAxis(
                        ap=idx_sb[:, g * G * J : (g + 1) * G * J], axis=0
                    ),
                )
                for k in range(G):
                    t = g * G + k
                    base = k * J * D
                    embt = emb[:, base : base + J * D]
                    emb_in = smallp.tile([P, D], BF16, tag="embin")
                    nc.vector.tensor_mul(
                        emb_in[:], embt[:, 0:D], mask_sb[:, t * D : (t + 1) * D]
                    )
                    prods = workp.tile([P, 16 * D], BF16, tag="prods")
                    p3 = prods[:].rearrange("p (j d) -> p j d", d=D)
                    nc.vector.tensor_tensor(
                        out=p3,
                        in0=embt[:, D : J * D].rearrange("p (j d) -> p j d", d=D),
                        in1=emb_in[:].unsqueeze(1).broadcast_to((P, 16, D)),
                        op=mybir.AluOpType.mult,
                    )
                    f1 = workp.tile([P, 16 * 64], BF16, tag="f1")
                    f1v = f1[:].rearrange("p (j d) -> p j d", d=64)
                    nc.vector.tensor_add(f1v, p3[:, :, 0:64], p3[:, :, 64:128])
                    f2 = workp.tile([P, 16 * 32], BF16, tag="f2")
                    f2v = f2[:].rearrange("p (j d) -> p j d", d=32)
                    nc.vector.tensor_add(f2v, f1v[:, :, 0:32], f1v[:, :, 32:64])
                    f3 = workp.tile([P, 16 * 16], BF16, tag="f3")
                    f3v = f3[:].rearrange("p (j d) -> p j d", d=16)
                    nc.vector.tensor_add(f3v, f2v[:, :, 0:16], f2v[:, :, 16:32])
                    scores = scores_all[:, t * 16 : (t + 1) * 16]
                    nc.vector.tensor_reduce(
                        scores, f3v, axis=mybir.AxisListType.X, op=mybir.AluOpType.add
                    )
                    nc.scalar.activation(
                        sig_all[:, t * 16 : (t + 1) * 16],
                        scores,
                        mybir.ActivationFunctionType.Sigmoid,
                        scale=-1.0,
                    )

            # ln(sigmoid(-x)) = -softplus(x); accumulate per chunk
            chunk = T * 16 // LN_CHUNKS
            for c in range(LN_CHUNKS):
                nc.scalar.activation(
                    ln_scratch[:],
                    sig_all[:, c * chunk : (c + 1) * chunk],
                    mybir.ActivationFunctionType.Ln,
                    accum_out=tsum[:, c : c + 1],
                )

            total = constp.tile([P, 1], F32)
            nc.vector.tensor_reduce(
                total[:], tsum[:], axis=mybir.AxisListType.X, op=mybir.AluOpType.add
            )
            nc.sync.dma_start(out[:], total[:])
    nc.compile()
    return nc


def _get_nc():
    if "nc" not in _CACHE:
        _CACHE["nc"] = _build_nc()
    return _CACHE["nc"]


def kernel(target, context, neg_idx, dropout_mask, W_target, W_context):
    global LAST_RESULT
    nc = _get_nc()

    target = np.asarray(target).astype(np.int32, copy=False)
    context = np.asarray(context).astype(np.int32, copy=False)
    neg_idx = np.asarray(neg_idx).astype(np.int32, copy=False)
    dropout_mask = np.asarray(dropout_mask, dtype=np.float32)
    W_target = np.asarray(W_target, dtype=np.float32)
    W_context = np.asarray(W_context, dtype=np.float32)

    w_cat = np.ascontiguousarray(
        np.concatenate([W_target, W_context], axis=0).astype(NPBF16)
    )
    idx_cat = np.empty((B, J), np.int32)
    idx_cat[:, 0] = target
    idx_cat[:, 1] = context + V
    idx_cat[:, 2:] = neg_idx + V
    mask_bf = dropout_mask.astype(NPBF16)

    in_maps = []
    for c in range(NCORES):
        sl = slice(c * BLOC, (c + 1) * BLOC)
        idxs = np.ascontiguousarray(
            idx_cat[sl].reshape(T, P, J).transpose(1, 0, 2).reshape(P, T * J)
        )
        maskr = np.ascontiguousarray(
            mask_bf[sl].reshape(T, P, D).transpose(1, 0, 2).reshape(P, T * D)
        )
        in_maps.append({"w_cat": w_cat, "idx": idxs, "maskr": maskr})

    trace = bool(int(os.environ.get("KERNEL_TRACE", "0")))
    res = bass_utils.run_bass_kernel_spmd(
        nc, in_maps, core_ids=list(range(NCORES)), trace=trace
    )
    LAST_RESULT = res

    tot = 0.0
    for r in res.results:
        tot += float(r["out"].astype(np.float64).sum())
    # device accumulated sum of ln(sigmoid(-x)) = -sum of softplus(x)
    loss = -tot / B
    return np.asarray(np.float32(loss))
